# revision 6
# baseline (speedup 1.0000x reference)
"""Self-contained Trainium2 Bass kernel for the 2-layer GAT (EnhancedGAT).

Strategy (graph/data parallel over 8 NeuronCores):
  - Nodes are partitioned by destination: core c owns nodes [c*6250, (c+1)*6250).
  - Host adds self-loops, sorts edges by dst into 49 blocks of 128 dst nodes
    per core. Within each block, edges are split by source id into a lo
    (src < 32768) and hi section so the int16-indexed dma_gather can address
    the full node table via two base slices; each section is padded with
    index-0 dummy edges (dst_local = -1) to uniform tile counts so one SPMD
    program serves all cores.
  - Per layer, each core computes its shard of the bf16 message table
    [h | a_src.h | a_dst.h | pad] with PE matmuls against host-precomputed
    fused weights, then an AllGather replicates the table.
  - Edge phase per dst block: two dma_gathers pull the 512B/256B source rows;
    dst-side logits come from a direct load of the block's own table rows
    expanded per-edge with a one-hot-transpose matmul; softmax runs without
    max-subtraction (bounded logits), so the weighted mean is
    sum(ee*h_src)/sum(ee), both accumulated by one PE matmul per 128-edge
    tile against a one-hot dst matrix built with broadcast is_equal compares.
"""

import sys

if "/opt/trn_rl_repo" not in sys.path:
    sys.path.insert(0, "/opt/trn_rl_repo")

import numpy as np
from ml_dtypes import bfloat16

P = 128
NCORES = 8
N = 50000
NPC = N // NCORES            # 6250 nodes per core
NB = (NPC + P - 1) // P      # 49 dst blocks per core (last has 106 rows)
FIN = 256
HID = 32
HEADS = 4
OUTD = 64
NEG = 0.2
HALF = 32768                 # int16 gather split point
R1 = 256                     # L1 table row (bf16): h1(128)|als1(4)|ald1(4)|pad
MC1, NH1 = 128, 4
R2 = 128                     # L2 table row (bf16): h2(64)|als2(1)|ald2(1)|pad
MC2, NH2 = 64, 1
B = 8                        # edge tiles per DVE/ACT batch

_cache = {}


def _preprocess(x, edge_index, W1, att_src1, att_dst1, b1, W2, att_src2,
                att_dst2, b2):
    x = np.asarray(x, np.float32)
    W1 = np.asarray(W1, np.float32)
    W2 = np.asarray(W2, np.float32)
    att_src1 = np.asarray(att_src1, np.float32)
    att_dst1 = np.asarray(att_dst1, np.float32)
    att_src2 = np.asarray(att_src2, np.float32)
    att_dst2 = np.asarray(att_dst2, np.float32)
    b1 = np.asarray(b1, np.float32)
    b2 = np.asarray(b2, np.float32)

    # fused weights: [W | W@Asrc | W@Adst]
    A1s = np.zeros((HEADS * HID, HEADS), np.float32)
    A1d = np.zeros((HEADS * HID, HEADS), np.float32)
    for h in range(HEADS):
        A1s[h * HID:(h + 1) * HID, h] = att_src1[h]
        A1d[h * HID:(h + 1) * HID, h] = att_dst1[h]
    w1c = np.concatenate([W1, W1 @ A1s, W1 @ A1d], axis=1)          # [256,136]
    w2c = np.concatenate(
        [W2, W2 @ att_src2[0][:, None], W2 @ att_dst2[0][:, None]], axis=1
    )                                                               # [128,66]
    b1b = np.broadcast_to(b1, (P, HEADS * HID)).copy()
    b2b = np.broadcast_to(b2, (P, OUTD)).copy()
    iota_row = np.broadcast_to(np.arange(P, dtype=np.int8), (P, P)).copy()
    iota_p = np.arange(P, dtype=np.int8).reshape(P, 1).copy()
    ident = np.eye(P, dtype=np.float32)

    # ---- edges: self-loops, sort by dst, bucket per (core, block, lo/hi) ----
    loops = np.arange(N, dtype=np.int64)
    src = np.concatenate([np.asarray(edge_index[0], np.int64), loops])
    dst = np.concatenate([np.asarray(edge_index[1], np.int64), loops])
    hi_flag = (src >= HALF).astype(np.int64)
    core = dst // NPC
    local = dst - core * NPC
    nbk = local // P
    dloc = local - nbk * P
    # sort by (core, block, hi_flag); order within a bucket irrelevant
    key = ((core * NB + nbk) * 2 + hi_flag)
    order = np.argsort(key, kind="stable")
    src_s = src[order].astype(np.int32)
    key_s = key[order]
    dloc_s = dloc[order].astype(np.int8)
    hi_s = hi_flag[order]
    Etot = src_s.shape[0]

    counts = np.bincount(key_s, minlength=NCORES * NB * 2)
    starts = np.zeros(NCORES * NB * 2 + 1, np.int64)
    np.cumsum(counts, out=starts[1:])
    lo_counts = counts[0::2]
    hi_counts = counts[1::2]
    T_lo = int((lo_counts.max() + P - 1) // P)
    T_hi = int((hi_counts.max() + P - 1) // P)
    TB = T_lo + T_hi

    pos = np.arange(Etot, dtype=np.int64) - starts[key_s]

    # per (core*NB) block: idx slots [T_lo*128] + [T_hi*128]; dloc [TB*128]
    nblk = NCORES * NB
    idx_lo = np.zeros((nblk, T_lo * P), np.int16)
    idx_hi = np.zeros((nblk, T_hi * P), np.int16)
    dloc_all = np.full((nblk, TB * P), -1, np.int8)
    blk = key_s // 2
    lo_m = hi_s == 0
    hi_m = ~lo_m
    idx_lo[blk[lo_m], pos[lo_m]] = src_s[lo_m].astype(np.int16)
    idx_hi[blk[hi_m], pos[hi_m]] = (src_s[hi_m] - HALF).astype(np.int16)
    dloc_all[blk[lo_m], pos[lo_m]] = dloc_s[lo_m]
    dloc_all[blk[hi_m], T_lo * P + pos[hi_m]] = dloc_s[hi_m]

    def wrap(a, T):
        # [NCORES, NB, T*128] -> int16 wrapped [NCORES, 128, NB*T*8]
        a = a.reshape(NCORES, NB, T * 8, 16)
        a = a.transpose(0, 3, 1, 2).reshape(NCORES, 16, NB * T * 8)
        return np.ascontiguousarray(np.tile(a, (1, 8, 1)))

    idx_lo_w = wrap(idx_lo, T_lo)
    idx_hi_w = wrap(idx_hi, T_hi)
    # dstloc lanes: edge j of tile c -> partition j%128 => [tiles,128].T
    dl = dloc_all.reshape(NCORES, NB, TB, P)
    dstloc = np.ascontiguousarray(
        dl.transpose(0, 3, 1, 2).reshape(NCORES, P, NB * TB))
    dstlocT = np.ascontiguousarray(
        dloc_all.reshape(NCORES, 1, NB * TB * P))

    in_maps = []
    for c in range(NCORES):
        xs = np.ascontiguousarray(x[c * NPC:(c + 1) * NPC].T)       # [256,6250]
        in_maps.append({
            "xs": xs,
            "idxlo": idx_lo_w[c],
            "idxhi": idx_hi_w[c],
            "dstloc": dstloc[c],
            "dstlocT": dstlocT[c],
            "w1c": w1c,
            "w2c": w2c,
            "b1b": b1b,
            "b2b": b2b,
            "iotar": iota_row,
            "iotap": iota_p,
            "ident": ident,
        })
    return in_maps, T_lo, T_hi


def _build(T_lo, T_hi):
    from concourse import bacc, bass, mybir, tile
    from concourse.library_config import mlp

    f32 = mybir.dt.float32
    bf16 = mybir.dt.bfloat16
    i16 = mybir.dt.int16
    i8 = mybir.dt.int8
    EQ = mybir.AluOpType.is_equal
    ADD = mybir.AluOpType.add
    MULT = mybir.AluOpType.mult
    MAX = mybir.AluOpType.max
    EXP = mybir.ActivationFunctionType.Exp
    TB = T_lo + T_hi
    NLO, NHI = T_lo * P, T_hi * P

    nc = bacc.Bacc("TRN2", target_bir_lowering=False, debug=False,
                   num_devices=NCORES)

    def inp(name, shape, dt):
        return nc.dram_tensor(name, shape, dt, kind="ExternalInput").ap()

    xs = inp("xs", [FIN, NPC], f32)
    idxlo = inp("idxlo", [P, NB * T_lo * 8], i16)
    idxhi = inp("idxhi", [P, NB * T_hi * 8], i16)
    dstloc = inp("dstloc", [P, NB * TB], i8)
    dstlocT = inp("dstlocT", [1, NB * TB * P], i8)
    w1c = inp("w1c", [FIN, MC1 + 2 * NH1], f32)
    w2c = inp("w2c", [P, MC2 + 2 * NH2], f32)
    b1b = inp("b1b", [P, HEADS * HID], f32)
    b2b = inp("b2b", [P, OUTD], f32)
    iotar = inp("iotar", [P, P], i8)
    iotap = inp("iotap", [P, 1], i8)
    ident = inp("ident", [P, P], f32)
    out = nc.dram_tensor("out", [NPC, OUTD], f32, kind="ExternalOutput").ap()

    with tile.TileContext(nc) as tc:
        with (
            tc.tile_pool(name="dram", bufs=1, space="DRAM") as dram,
            tc.tile_pool(name="meta", bufs=1) as meta,
            tc.tile_pool(name="psum", bufs=1, space="PSUM") as psp,
            tc.tile_pool(name="work", bufs=1) as wp,
        ):
            h1loc = dram.tile([NPC, R1], bf16)
            h1full = dram.tile([N, R1], bf16, addr_space="Shared")
            h2loc = dram.tile([NPC, R2], bf16)
            h2full = dram.tile([N, R2], bf16, addr_space="Shared")

            nc.gpsimd.load_library(mlp)

            # ---------------- persistent SBUF constants ----------------
            idxlo_t = meta.tile([P, NB * T_lo * 8], i16)
            nc.sync.dma_start(out=idxlo_t[:], in_=idxlo[:])
            idxhi_t = meta.tile([P, NB * T_hi * 8], i16)
            nc.sync.dma_start(out=idxhi_t[:], in_=idxhi[:])
            dstloc_t = meta.tile([P, NB * TB], i8)
            nc.sync.dma_start(out=dstloc_t[:], in_=dstloc[:])
            iotar_t = meta.tile([P, P], i8)
            nc.sync.dma_start(out=iotar_t[:], in_=iotar[:])
            iotap_t = meta.tile([P, 1], i8)
            nc.sync.dma_start(out=iotap_t[:], in_=iotap[:])
            ident_t = meta.tile([P, P], f32)
            nc.sync.dma_start(out=ident_t[:], in_=ident[:])
            w2c_t = meta.tile([P, MC2 + 2 * NH2], f32)
            nc.sync.dma_start(out=w2c_t[:], in_=w2c[:])
            b1b_t = meta.tile([P, HEADS * HID], f32)
            nc.sync.dma_start(out=b1b_t[:], in_=b1b[:])
            b2b_t = meta.tile([P, OUTD], f32)
            nc.sync.dma_start(out=b2b_t[:], in_=b2b[:])

            # ---------------- phase A: h1|als1|ald1 for own nodes ----------------
            with tc.tile_pool(name="pA", bufs=1) as pa:
                xs0 = pa.tile([P, NPC], f32)
                nc.sync.dma_start(out=xs0[:], in_=xs[0:P, :])
                xs1 = pa.tile([P, NPC], f32)
                nc.sync.dma_start(out=xs1[:], in_=xs[P:FIN, :])
                w10 = pa.tile([P, MC1 + 2 * NH1], f32)
                nc.sync.dma_start(out=w10[:], in_=w1c[0:P, :])
                w11 = pa.tile([P, MC1 + 2 * NH1], f32)
                nc.sync.dma_start(out=w11[:], in_=w1c[P:FIN, :])
                for nb in range(NB):
                    c0 = nb * P
                    rows = min(P, NPC - c0)
                    pp = psp.tile([P, MC1 + 2 * NH1], f32, tag="agg", bufs=2,
                                  name=f"ppA_{nb}")
                    nc.tensor.matmul(out=pp[:rows], lhsT=xs0[:, c0:c0 + rows],
                                     rhs=w10[:], start=True, stop=False)
                    nc.tensor.matmul(out=pp[:rows], lhsT=xs1[:, c0:c0 + rows],
                                     rhs=w11[:], start=False, stop=True)
                    h1row = pa.tile([P, R1], bf16, tag="h1row", bufs=3,
                                    name=f"h1row_{nb}")
                    nc.vector.tensor_copy(out=h1row[:rows, 0:MC1 + 2 * NH1],
                                          in_=pp[:rows])
                    nc.sync.dma_start(
                        out=h1loc[c0:c0 + rows, 0:MC1 + 2 * NH1],
                        in_=h1row[:rows, 0:MC1 + 2 * NH1])

            nc.gpsimd.collective_compute(
                "AllGather", mybir.AluOpType.bypass,
                replica_groups=[list(range(NCORES))],
                ins=[h1loc.opt()], outs=[h1full.opt()],
            )

            # ---------------- shared edge phase ----------------
            def edge_phase(lname, table, loctbl, Rb, MC, NH, postproc):
                HD = MC // NH
                for nb in range(NB):
                    rows = min(P, NPC - nb * P)
                    sfb = f"{lname}_{nb}"
                    gb = wp.tile([P, TB, Rb], bf16, tag="gb", bufs=2,
                                 name=f"gb_{sfb}")
                    # >=2048 idxs per dma_gather wedges the device (SWDGE
                    # ring limit); chunk to 8 tiles = 1024 idxs.
                    for c0 in range(0, T_lo, B):
                        cw = min(B, T_lo - c0)
                        nc.gpsimd.dma_gather(
                            gb[:, c0:c0 + cw, :], table[0:HALF, :],
                            idxlo_t[:, nb * T_lo * 8 + c0 * 8:
                                    nb * T_lo * 8 + (c0 + cw) * 8],
                            cw * P, cw * P, Rb)
                    for c0 in range(0, T_hi, B):
                        cw = min(B, T_hi - c0)
                        nc.gpsimd.dma_gather(
                            gb[:, T_lo + c0:T_lo + c0 + cw, :], table[HALF:N, :],
                            idxhi_t[:, nb * T_hi * 8 + c0 * 8:
                                    nb * T_hi * 8 + (c0 + cw) * 8],
                            cw * P, cw * P, Rb)
                    aldb = wp.tile([P, NH1], bf16, tag="aldb", bufs=2,
                                   name=f"aldb_{sfb}")
                    if rows < P:
                        nc.vector.memset(aldb[:], 0)
                    nc.sync.dma_start(
                        out=aldb[:rows, 0:NH],
                        in_=loctbl[nb * P:nb * P + rows, MC + NH:MC + 2 * NH])
                    pacc = psp.tile([P, NH1 + MC1], f32, tag="agg", bufs=2,
                                    name=f"pacc_{sfb}")
                    for w0 in range(0, TB, B):
                        Bw = min(B, TB - w0)
                        sfx = f"{lname}_{nb}_{w0}"
                        bct = wp.tile([P, B * P], i8, tag="bct", bufs=3,
                                      name=f"bct_{sfx}")
                        src_ap = bass.AP(
                            dstlocT.tensor,
                            dstlocT.offset + (nb * TB + w0) * P,
                            [[0, P], [1, Bw * P]])
                        nc.sync.dma_start(out=bct[:, 0:Bw * P], in_=src_ap)
                        s4 = wp.tile([P, B, P], bf16, tag="s4", bufs=3,
                                     name=f"s4_{sfx}")
                        dsl = dstloc_t[:, nb * TB + w0:nb * TB + w0 + Bw]
                        nc.vector.tensor_tensor(
                            out=s4[:, 0:Bw, :],
                            in0=bass.AP(dsl.tensor, dsl.offset,
                                        [dsl.ap[0], [1, Bw], [0, P]]),
                            in1=bass.AP(iotar_t.tensor, iotar_t.offset,
                                        [iotar_t.ap[0], [0, Bw], [1, P]]),
                            op=EQ)
                        sT4 = wp.tile([P, B, P], bf16, tag="sT4", bufs=3,
                                      name=f"sT4_{sfx}")
                        nc.vector.tensor_tensor(
                            out=sT4[:, 0:Bw, :],
                            in0=bass.AP(iotap_t.tensor, iotap_t.offset,
                                        [iotap_t.ap[0], [0, Bw], [0, P]]),
                            in1=bass.AP(bct.tensor, bct.offset,
                                        [bct.ap[0], [P, Bw], [1, P]]),
                            op=EQ)
                        alde = psp.tile([P, B * NH1], f32, tag="alde", bufs=2,
                                        name=f"alde_{sfx}")
                        for j in range(Bw):
                            nc.tensor.matmul(
                                out=alde[:, j * NH:(j + 1) * NH],
                                lhsT=sT4[:, j, :], rhs=aldb[:, 0:NH],
                                start=True, stop=True)
                        e1 = wp.tile([P, B, NH1], f32, tag="e1", bufs=3,
                                     name=f"e1_{sfx}")
                        nc.vector.tensor_tensor(
                            out=e1[:, 0:Bw, 0:NH],
                            in0=gb[:, w0:w0 + Bw, MC:MC + NH],
                            in1=bass.AP(alde.tensor, alde.offset,
                                        [alde.ap[0], [NH, Bw], [1, NH]]),
                            op=ADD)
                        e2 = wp.tile([P, B, NH1], f32, tag="e2", bufs=3,
                                     name=f"e2_{sfx}")
                        nc.vector.tensor_scalar_mul(
                            out=e2[:, 0:Bw, 0:NH], in0=e1[:, 0:Bw, 0:NH],
                            scalar1=NEG)
                        e3 = wp.tile([P, B, NH1], f32, tag="e3", bufs=3,
                                     name=f"e3_{sfx}")
                        nc.vector.tensor_tensor(
                            out=e3[:, 0:Bw, 0:NH], in0=e1[:, 0:Bw, 0:NH],
                            in1=e2[:, 0:Bw, 0:NH], op=MAX)
                        eerep = wp.tile([P, B, MC1], bf16, tag="eerep", bufs=3,
                                        name=f"eerep_{sfx}")
                        nc.scalar.activation(
                            out=eerep[:, 0:Bw, 0:MC],
                            in_=bass.AP(e3.tensor, e3.offset,
                                        [e3.ap[0], [NH1, Bw], [1, NH], [0, HD]]),
                            func=EXP)
                        rhs = wp.tile([P, B, NH1 + MC1], bf16, tag="rhs",
                                      bufs=3, name=f"rhs_{sfx}")
                        nc.vector.tensor_tensor(
                            out=rhs[:, 0:Bw, NH:NH + MC],
                            in0=gb[:, w0:w0 + Bw, 0:MC],
                            in1=eerep[:, 0:Bw, 0:MC], op=MULT)
                        nc.vector.tensor_copy(
                            out=rhs[:, 0:Bw, 0:NH],
                            in_=bass.AP(eerep.tensor, eerep.offset,
                                        [eerep.ap[0], [MC1, Bw], [HD, NH]]))
                        for j in range(Bw):
                            t = w0 + j
                            nc.tensor.matmul(
                                out=pacc[:, 0:NH + MC],
                                lhsT=s4[:, j, :], rhs=rhs[:, j, 0:NH + MC],
                                start=(t == 0), stop=(t == TB - 1))
                    postproc(nb, pacc, rows)

            # ---------------- layer-1 postprocess -> h2 table rows ----------------
            def post1(nb, pacc, rows):
                sfx = f"p1_{nb}"
                dn = wp.tile([P, NH1], f32, tag="dn", bufs=2, name=f"dn_{sfx}")
                nc.vector.tensor_scalar_add(out=dn[:], in0=pacc[:, 0:NH1],
                                            scalar1=1e-30)
                rc = wp.tile([P, NH1], f32, tag="rc", bufs=2, name=f"rc_{sfx}")
                nc.vector.reciprocal(out=rc[:], in_=dn[:])
                o1 = wp.tile([P, HEADS * HID], f32, tag="o1", bufs=2,
                             name=f"o1_{sfx}")
                for h in range(HEADS):
                    nc.vector.tensor_scalar_mul(
                        out=o1[:, h * HID:(h + 1) * HID],
                        in0=pacc[:, NH1 + h * HID:NH1 + (h + 1) * HID],
                        scalar1=rc[:, h:h + 1])
                o1b = wp.tile([P, HEADS * HID], f32, tag="o1b", bufs=2,
                              name=f"o1b_{sfx}")
                nc.vector.tensor_tensor(out=o1b[:], in0=o1[:], in1=b1b_t[:],
                                        op=ADD)
                # ELU(x) = max(x,0) + exp(min(x,0)) - 1
                mn = wp.tile([P, HEADS * HID], f32, tag="mn", bufs=2,
                             name=f"mn_{sfx}")
                nc.vector.tensor_scalar_min(out=mn[:], in0=o1b[:], scalar1=0.0)
                ex = wp.tile([P, HEADS * HID], f32, tag="ex", bufs=2,
                             name=f"ex_{sfx}")
                nc.scalar.activation(out=ex[:], in_=mn[:], func=EXP)
                mx = wp.tile([P, HEADS * HID], f32, tag="mx", bufs=2,
                             name=f"mx_{sfx}")
                nc.vector.tensor_scalar_max(out=mx[:], in0=o1b[:], scalar1=0.0)
                sm = wp.tile([P, HEADS * HID], f32, tag="sm", bufs=2,
                             name=f"sm_{sfx}")
                nc.vector.tensor_tensor(out=sm[:], in0=mx[:], in1=ex[:], op=ADD)
                h1e = wp.tile([P, HEADS * HID], f32, tag="h1e", bufs=2,
                              name=f"h1e_{sfx}")
                nc.vector.tensor_scalar_add(out=h1e[:], in0=sm[:], scalar1=-1.0)
                pt = psp.tile([P, P], f32, tag="tp", bufs=2, name=f"pt_{sfx}")
                nc.tensor.transpose(out=pt[:], in_=h1e[:], identity=ident_t[:])
                h1eT = wp.tile([P, P], f32, tag="h1eT", bufs=2,
                               name=f"h1eT_{sfx}")
                nc.vector.tensor_copy(out=h1eT[:], in_=pt[:])
                p2 = psp.tile([P, MC2 + 2 * NH2], f32, tag="h2", bufs=2,
                              name=f"p2_{sfx}")
                nc.tensor.matmul(out=p2[:], lhsT=h1eT[:], rhs=w2c_t[:],
                                 start=True, stop=True)
                h2row = wp.tile([P, R2], bf16, tag="h2row", bufs=2,
                                name=f"h2row_{sfx}")
                nc.vector.tensor_copy(out=h2row[:, 0:MC2 + 2 * NH2], in_=p2[:])
                nc.sync.dma_start(
                    out=h2loc[nb * P:nb * P + rows, 0:MC2 + 2 * NH2],
                    in_=h2row[:rows, 0:MC2 + 2 * NH2])

            edge_phase("L1", h1full, h1loc, R1, MC1, NH1, post1)

            nc.gpsimd.collective_compute(
                "AllGather", mybir.AluOpType.bypass,
                replica_groups=[list(range(NCORES))],
                ins=[h2loc.opt()], outs=[h2full.opt()],
            )

            # ---------------- layer-2 postprocess -> final output ----------------
            def post2(nb, pacc, rows):
                sfx = f"p2_{nb}"
                dn = wp.tile([P, NH1], f32, tag="dn", bufs=2, name=f"dn_{sfx}")
                nc.vector.tensor_scalar_add(out=dn[:, 0:1], in0=pacc[:, 0:1],
                                            scalar1=1e-30)
                rc = wp.tile([P, NH1], f32, tag="rc", bufs=2, name=f"rc_{sfx}")
                nc.vector.reciprocal(out=rc[:, 0:1], in_=dn[:, 0:1])
                o2 = wp.tile([P, OUTD], f32, tag="o1", bufs=2, name=f"o2_{sfx}")
                nc.vector.tensor_scalar_mul(out=o2[:], in0=pacc[:, 1:1 + OUTD],
                                            scalar1=rc[:, 0:1])
                o2b = wp.tile([P, OUTD], f32, tag="o1b", bufs=2,
                              name=f"o2b_{sfx}")
                nc.vector.tensor_tensor(out=o2b[:], in0=o2[:], in1=b2b_t[:],
                                        op=ADD)
                nc.sync.dma_start(out=out[nb * P:nb * P + rows, :],
                                  in_=o2b[:rows])

            edge_phase("L2", h2full, h2loc, R2, MC2, NH2, post2)

    nc.compile()
    return nc


def _get_nc(T_lo, T_hi):
    key = (T_lo, T_hi)
    if key not in _cache:
        _cache[key] = _build(T_lo, T_hi)
    return _cache[key]


def kernel(**inputs):
    from concourse import bass_utils

    in_maps, T_lo, T_hi = _preprocess(**inputs)
    nc = _get_nc(T_lo, T_hi)
    res = bass_utils.run_bass_kernel_spmd(nc, in_maps,
                                          core_ids=list(range(NCORES)))
    outs = [res.results[c]["out"] for c in range(NCORES)]
    return np.concatenate(outs, axis=0)


def kernel_traced(**inputs):
    """Like kernel() but with NTFF profiling; returns (output, BassKernelResults)."""
    from concourse import bass_utils

    in_maps, T_lo, T_hi = _preprocess(**inputs)
    nc = _get_nc(T_lo, T_hi)
    res = bass_utils.run_bass_kernel_spmd(nc, in_maps,
                                          core_ids=list(range(NCORES)),
                                          trace=True)
    outs = [res.results[c]["out"] for c in range(NCORES)]
    return np.concatenate(outs, axis=0), res
